# revision 9
# baseline (speedup 1.0000x reference)
"""Trainium2 Bass kernel for DemandAwareCrossAttention.

Reference computation (per pixel, fully pointwise in (H, W)):
    enc  = w_d2 @ relu(w_d1 @ demand + b_d1) + b_d2
    qs   = ego + enc + pos
    q    = (wq @ qs + bq)   reshaped [8 heads, 32]
    k_n  = wk @ collab_n + bk ; v_n = wv @ collab_n + bv     (n = 0..3)
    s_nm = q_m . k_nm / sqrt(32)
    a    = softmax_n(s)
    u    = sum_n a_nm * v_n            -> [256]
    out  = wo @ u + bo

Sharding: split W across the 8 cores (4096 pixels each); weights replicated.

Device layout ("layout A"): channels on SBUF partitions, pixels on the free
dim, channel chunks c in {0,1} of 128.  Per 256-pixel tile:
  - all 1x1 convs are PE matmuls (bf16, fp32 PSUM accumulate)
  - scores: DVE q*k product, then a masked matmul sums over d within each
    head -> scores for collab n land on PSUM partitions 32n+h (heads 4c+h)
  - softmax over n without any divide: e = exp(s) (ScalarE), denom via a
    masked matmul, L = ln(denom) written into spare rows of the score tile,
    then one masked matmul forms z = s - L broadcast over d, a = exp(z)
  - combine: DVE  u = sum_n a_n * v_n ; out projection on PE.

Bias handling (free): b_d1 rides the relu's bias slot; bq (+ wq@b_d2) rides
the q PSUM->SBUF copy; bk only shifts all collabs' scores equally per head,
so it cancels in the softmax and is dropped; bv enters through sum_n a = 1
so wo@bv + bo rides the output copy.  q is pre-scaled by 1/sqrt(32) on host.
"""

import math
import os
import numpy as np
import ml_dtypes
from contextlib import ExitStack

import concourse.bass as bass
import concourse.tile as tile
from concourse import bacc, mybir
from concourse.bass import ts
from concourse.bass_utils import run_bass_kernel_spmd

BF = mybir.dt.bfloat16
F32 = mybir.dt.float32
AF = mybir.ActivationFunctionType

C = 256          # model dim
HID = 128        # demand-encoder hidden
NH = 8           # heads
HD = 32          # head dim
NCOL = 4         # collaborators
H, W = 128, 256
NCORES = 8
WSL = W // NCORES          # 32 columns of W per core
PPC = H * WSL              # 4096 pixels per core
TP = 256                   # pixels per tile
NT = PPC // TP             # 16 tiles


def _build_program(has_pos: bool) -> bass.Bass:
    nc = bacc.Bacc("TRN2", target_bir_lowering=False, debug=False)

    ego_d = nc.dram_tensor("ego", [128, 2, PPC], BF, kind="ExternalInput")
    dem_d = nc.dram_tensor("demand", [3, PPC], BF, kind="ExternalInput")
    col_d = nc.dram_tensor("collab", [NCOL, 128, 2, PPC], BF, kind="ExternalInput")
    if has_pos:
        pos_d = nc.dram_tensor("pos", [128, 2, PPC], BF, kind="ExternalInput")
    wd1T_d = nc.dram_tensor("wd1T", [3, HID], BF, kind="ExternalInput")
    wqd2T_d = nc.dram_tensor("wqd2T", [HID, C], BF, kind="ExternalInput")
    wqT_d = nc.dram_tensor("wqT", [2, 128, C], BF, kind="ExternalInput")
    wkT_d = nc.dram_tensor("wkT", [2, 128, C], BF, kind="ExternalInput")
    wvT_d = nc.dram_tensor("wvT", [2, 128, C], BF, kind="ExternalInput")
    woT_d = nc.dram_tensor("woT", [2, 128, C], BF, kind="ExternalInput")
    bd1_d = nc.dram_tensor("bd1", [HID, 1], F32, kind="ExternalInput")
    bq_d = nc.dram_tensor("bq", [128, 2], F32, kind="ExternalInput")
    bo_d = nc.dram_tensor("bo", [128, 2], F32, kind="ExternalInput")
    smask_d = nc.dram_tensor("smask", [128, 32], BF, kind="ExternalInput")
    dmask_d = nc.dram_tensor("dmask", [128, 4], BF, kind="ExternalInput")
    zmask_d = nc.dram_tensor("zmask", [NCOL, 128, 128], BF, kind="ExternalInput")
    out_d = nc.dram_tensor("out", [128, 2, PPC], F32, kind="ExternalOutput")

    with ExitStack() as ctx:
        tc = ctx.enter_context(tile.TileContext(nc))

        wp = ctx.enter_context(tc.tile_pool(name="wts", bufs=1))
        io = ctx.enter_context(tc.tile_pool(name="io", bufs=3))
        sp = ctx.enter_context(tc.tile_pool(name="sb", bufs=2))
        wvp = ctx.enter_context(tc.tile_pool(name="wv", bufs=2))
        pm = ctx.enter_context(tc.tile_pool(name="pm", bufs=3, space="PSUM"))
        pk = ctx.enter_context(tc.tile_pool(name="pk", bufs=2, space="PSUM"))
        pv = ctx.enter_context(tc.tile_pool(name="pv", bufs=2, space="PSUM"))

        # ---- load weights/masks once ----
        def _load(dram, shape, dtype, tag):
            t = wp.tile(shape, dtype, tag=tag)
            nc.sync.dma_start(out=t, in_=dram[:])
            return t

        wd1T = _load(wd1T_d, [3, HID], BF, "wd1T")
        wqd2T = _load(wqd2T_d, [HID, C], BF, "wqd2T")
        wqT = [_load(wqT_d[kc], [128, C], BF, f"wqT{kc}") for kc in range(2)]
        wkT = [_load(wkT_d[kc], [128, C], BF, f"wkT{kc}") for kc in range(2)]
        wvT = [_load(wvT_d[kc], [128, C], BF, f"wvT{kc}") for kc in range(2)]
        woT = [_load(woT_d[kc], [128, C], BF, f"woT{kc}") for kc in range(2)]
        bd1 = _load(bd1_d, [HID, 1], F32, "bd1")
        bq = _load(bq_d, [128, 2], F32, "bq")
        bo = _load(bo_d, [128, 2], F32, "bo")
        smask = _load(smask_d, [128, 32], BF, "smask")
        dmask = _load(dmask_d, [128, 4], BF, "dmask")
        zmask = [_load(zmask_d[n], [128, 128], BF, f"zmask{n}") for n in range(NCOL)]

        for t in range(NT):
            px = ts(t, TP)

            ego = io.tile([128, 2, TP], BF, tag="ego")
            nc.sync.dma_start(out=ego, in_=ego_d[:, :, px])
            dem = io.tile([3, TP], BF, tag="dem")
            nc.sync.dma_start(out=dem, in_=dem_d[:, px])
            col = []
            for n in range(NCOL):
                cn = io.tile([128, 2, TP], BF, tag=f"col{n}")
                nc.sync.dma_start(out=cn, in_=col_d[n, :, :, px])
                col.append(cn)
            if has_pos:
                pos = io.tile([128, 2, TP], BF, tag="pos")
                nc.sync.dma_start(out=pos, in_=pos_d[:, :, px])

            # ---- demand encoder hidden ----
            h_ps = pm.tile([HID, TP], F32, tag="m")
            nc.tensor.matmul(out=h_ps, lhsT=wd1T, rhs=dem, start=True, stop=True)
            h_sb = sp.tile([HID, TP], BF, tag="h")
            nc.scalar.activation(out=h_sb, in_=h_ps, func=AF.Relu, bias=bd1[:, 0:1])

            # ---- q projection (scaled); enc folded in via wqd2T ----
            q_ps = pm.tile([128, 2, TP], F32, tag="m")
            for c in range(2):
                mcols = ts(c, 128)
                nc.tensor.matmul(out=q_ps[:, c, :], lhsT=wqT[0][:, mcols],
                                 rhs=ego[:, 0, :], start=True, stop=False)
                nc.tensor.matmul(out=q_ps[:, c, :], lhsT=wqT[1][:, mcols],
                                 rhs=ego[:, 1, :], start=False, stop=False)
                if has_pos:
                    nc.tensor.matmul(out=q_ps[:, c, :], lhsT=wqT[0][:, mcols],
                                     rhs=pos[:, 0, :], start=False, stop=False)
                    nc.tensor.matmul(out=q_ps[:, c, :], lhsT=wqT[1][:, mcols],
                                     rhs=pos[:, 1, :], start=False, stop=False)
                nc.tensor.matmul(out=q_ps[:, c, :], lhsT=wqd2T[:, mcols],
                                 rhs=h_sb, start=False, stop=True)
            q_sb = sp.tile([128, 2, TP], BF, tag="q")
            for c in range(2):
                nc.scalar.activation(out=q_sb[:, c, :], in_=q_ps[:, c, :],
                                     func=AF.Identity, bias=bq[:, c:c + 1])

            # ---- scores ----
            s_ps = pm.tile([128, 2, TP], F32, tag="m")
            for n in range(NCOL):
                k_ps = pk.tile([128, 2, TP], F32, tag="k")
                for c in range(2):
                    mcols = ts(c, 128)
                    nc.tensor.matmul(out=k_ps[:, c, :], lhsT=wkT[0][:, mcols],
                                     rhs=col[n][:, 0, :], start=True, stop=False)
                    nc.tensor.matmul(out=k_ps[:, c, :], lhsT=wkT[1][:, mcols],
                                     rhs=col[n][:, 1, :], start=False, stop=True)
                t_sb = sp.tile([128, 2, TP], BF, tag="t")
                nc.vector.tensor_mul(t_sb, q_sb, k_ps)
                for c in range(2):
                    nc.tensor.matmul(out=s_ps[32 * n:32 * n + 32, c, :], lhsT=smask,
                                     rhs=t_sb[:, c, :], start=True, stop=True,
                                     tile_position=(0, 32 * n))

            # ---- softmax over n (divide-free) ----
            e_sb = sp.tile([128, 2, TP], BF, tag="e")
            nc.scalar.activation(out=e_sb, in_=s_ps, func=AF.Exp)
            d_ps = pm.tile([4, 2, TP], F32, tag="m")
            for c in range(2):
                nc.tensor.matmul(out=d_ps[:, c, :], lhsT=dmask, rhs=e_sb[:, c, :],
                                 start=True, stop=True)
            s_sb = sp.tile([128, 2, TP], BF, tag="s")
            nc.scalar.activation(out=s_sb, in_=s_ps, func=AF.Copy)
            for c in range(2):
                nc.scalar.activation(out=s_sb[0:4, c, :], in_=d_ps[:, c, :],
                                     func=AF.Ln)

            # ---- weighted combine ----
            w_sb = []
            for n in range(NCOL):
                z_ps = pk.tile([128, 2, TP], F32, tag="k")
                for c in range(2):
                    nc.tensor.matmul(out=z_ps[:, c, :], lhsT=zmask[n],
                                     rhs=s_sb[:, c, :], start=True, stop=True)
                a_sb = sp.tile([128, 2, TP], BF, tag="a")
                nc.scalar.activation(out=a_sb, in_=z_ps, func=AF.Exp)
                v_ps = pv.tile([128, 2, TP], F32, tag="v")
                for c in range(2):
                    mcols = ts(c, 128)
                    nc.tensor.matmul(out=v_ps[:, c, :], lhsT=wvT[0][:, mcols],
                                     rhs=col[n][:, 0, :], start=True, stop=False)
                    nc.tensor.matmul(out=v_ps[:, c, :], lhsT=wvT[1][:, mcols],
                                     rhs=col[n][:, 1, :], start=False, stop=True)
                w_n = wvp.tile([128, 2, TP], BF, tag=f"w{n}")
                nc.vector.tensor_mul(w_n, a_sb, v_ps)
                w_sb.append(w_n)
            u01 = sp.tile([128, 2, TP], BF, tag="u01")
            nc.vector.tensor_add(u01, w_sb[0], w_sb[1])
            u23 = sp.tile([128, 2, TP], BF, tag="u23")
            nc.vector.tensor_add(u23, w_sb[2], w_sb[3])
            u = sp.tile([128, 2, TP], BF, tag="u")
            nc.vector.tensor_add(u, u01, u23)

            # ---- output projection ----
            o_ps = pm.tile([128, 2, TP], F32, tag="m")
            for c in range(2):
                mcols = ts(c, 128)
                nc.tensor.matmul(out=o_ps[:, c, :], lhsT=woT[0][:, mcols],
                                 rhs=u[:, 0, :], start=True, stop=False)
                nc.tensor.matmul(out=o_ps[:, c, :], lhsT=woT[1][:, mcols],
                                 rhs=u[:, 1, :], start=False, stop=True)
            o_sb = sp.tile([128, 2, TP], F32, tag="o")
            for c in range(2):
                nc.scalar.activation(out=o_sb[:, c, :], in_=o_ps[:, c, :],
                                     func=AF.Identity, bias=bo[:, c:c + 1])
            nc.sync.dma_start(out=out_d[:, :, px], in_=o_sb)

    if not nc.is_finalized():
        nc.finalize()
    return nc


_PROGRAMS: dict[bool, bass.Bass] = {}


def _get_program(has_pos: bool) -> bass.Bass:
    if has_pos not in _PROGRAMS:
        _PROGRAMS[has_pos] = _build_program(has_pos)
    return _PROGRAMS[has_pos]


def _bf16(x):
    return np.asarray(x, dtype=np.float32).astype(ml_dtypes.bfloat16)


def _shard_chw(x):
    """[C, H, W] fp32 -> per-core [128, 2, PPC] arrays (channel-chunked)."""
    xc = x.reshape(2, 128, H, W)
    out = []
    for i in range(NCORES):
        sl = xc[:, :, :, i * WSL:(i + 1) * WSL].reshape(2, 128, PPC)
        out.append(np.ascontiguousarray(sl.transpose(1, 0, 2)))
    return out


def _make_masks():
    # Scores for collab n, chunk-local head h live at PSUM/SBUF row 32n+4+h;
    # rows 0..3 of the score tile are later overwritten with L = ln(denom)
    # (32-aligned engine write), rows 32n+{0..3,8..31} stay exact zeros.
    smask = np.zeros((128, 32), np.float32)
    for h in range(4):
        smask[32 * h:32 * h + 32, 4 + h] = 1.0
    dmask = np.zeros((128, 4), np.float32)
    for n in range(NCOL):
        for h in range(4):
            dmask[32 * n + 4 + h, h] = 1.0
    zmask = np.zeros((NCOL, 128, 128), np.float32)
    for n in range(NCOL):
        for h in range(4):
            zmask[n, 32 * n + 4 + h, 32 * h:32 * h + 32] = 1.0
            zmask[n, h, 32 * h:32 * h + 32] -= 1.0
    return _bf16(smask), _bf16(dmask), _bf16(zmask)


def kernel(ego_features, ego_demand, collaborator_features,
           w_d1, b_d1, w_d2, b_d2, wq, bq, wk, bk, wv, bv, wo, bo,
           pos_emb):
    ego_features = np.asarray(ego_features, np.float32)
    ego_demand = np.asarray(ego_demand, np.float32)
    collaborator_features = np.asarray(collaborator_features, np.float32)
    w_d1 = np.asarray(w_d1, np.float32); b_d1 = np.asarray(b_d1, np.float32)
    w_d2 = np.asarray(w_d2, np.float32); b_d2 = np.asarray(b_d2, np.float32)
    wq = np.asarray(wq, np.float32); bq = np.asarray(bq, np.float32)
    wk = np.asarray(wk, np.float32); bk = np.asarray(bk, np.float32)
    wv = np.asarray(wv, np.float32); bv = np.asarray(bv, np.float32)
    wo = np.asarray(wo, np.float32); bo = np.asarray(bo, np.float32)
    pos_emb = np.asarray(pos_emb, np.float32)

    scale = 1.0 / math.sqrt(HD)
    wq_s = wq * scale
    wqd2 = wq_s @ w_d2                       # [C, HID]
    bq_eff = (bq + wq @ b_d2) * scale        # [C]
    bo_eff = bo + wo @ bv                    # [C]

    has_pos = bool(np.any(pos_emb))
    nc = _get_program(has_pos)

    smask, dmask, zmask = _make_masks()
    shared = {
        "wd1T": _bf16(w_d1.T),               # [3, HID]
        "wqd2T": _bf16(wqd2.T),              # [HID, C]
        "wqT": _bf16(wq_s.T.reshape(2, 128, C)),
        "wkT": _bf16(wk.T.reshape(2, 128, C)),
        "wvT": _bf16(wv.T.reshape(2, 128, C)),
        "woT": _bf16(wo.T.reshape(2, 128, C)),
        "bd1": np.ascontiguousarray(b_d1.reshape(HID, 1)),
        "bq": np.ascontiguousarray(bq_eff.reshape(2, 128).T),
        "bo": np.ascontiguousarray(bo_eff.reshape(2, 128).T),
        "smask": smask, "dmask": dmask, "zmask": zmask,
    }

    ego_sh = _shard_chw(_bf16(ego_features[0]))
    dem_full = _bf16(ego_demand[0])          # [3, H, W]
    col_sh = [_shard_chw(_bf16(collaborator_features[n])) for n in range(NCOL)]
    if has_pos:
        pos_sh = _shard_chw(_bf16(pos_emb[0]))

    in_maps = []
    for i in range(NCORES):
        m = dict(shared)
        m["ego"] = ego_sh[i]
        m["demand"] = np.ascontiguousarray(
            dem_full[:, :, i * WSL:(i + 1) * WSL].reshape(3, PPC))
        m["collab"] = np.stack([col_sh[n][i] for n in range(NCOL)])
        if has_pos:
            m["pos"] = pos_sh[i]
        in_maps.append(m)

    res = run_bass_kernel_spmd(nc, in_maps, list(range(NCORES)))

    out = np.empty((1, C, H, W), np.float32)
    for i in range(NCORES):
        oc = res.results[i]["out"]           # [128, 2, PPC]
        oc = oc.transpose(1, 0, 2).reshape(C, H, WSL)
        out[0, :, :, i * WSL:(i + 1) * WSL] = oc
    return out
